# revision 1
# baseline (speedup 1.0000x reference)
"""BrosSelfAttention Trainium2 kernel.

Problem: B=2, S=1024, H=768, NH=12, DH=64.
  q,k,v = proj(hidden);  scores = q@k.T + einsum('bnid,bijd->bnij', q, bbox)
  probs = softmax(scores/8 + mask);  ctx = probs@v

Sharding: 8 cores = (batch b in {0,1}) x (query block q4 in {0..3}, 256 rows).
Each core computes ctx for its 256 query rows, all 12 heads. K/V are
recomputed per core (full S for its batch).

Layout trick: scores are computed TRANSPOSED ([j, i] with j on partitions)
so that
  - the bbox term is computed as per-query-pair matmuls
    (stationary = bbox slice [sd=128, jj=128], moving = block-diag Q [128, 24])
    whose output [jj, (s,n)] has j on partitions,
  - the QK term ([jj, i] = kT_n^T @ qT_n) also has j on partitions,
  - the attention mask becomes a per-partition bias folded into the exp
    activation (exact reference semantics),
  - softmax row sums fall out of the PV matmul via a ones-column in V,
  - the PV matmul (lhsT = exp(scores^T) [jj, i], rhs = V [jj, 65]) contracts
    over j and lands ctx in [i, dh] layout directly.

All matmuls bf16 inputs with fp32 PSUM accumulation.
"""

import numpy as np
import ml_dtypes

import concourse.mybir as mybir
import concourse.tile as tile
from concourse import bacc
from concourse.bass_utils import run_bass_kernel_spmd

B, S, H, NH, DH = 2, 1024, 768, 12, 64
SQ = 256          # query rows per core
NC = 8            # cores
HC = H // 128     # 6 feature chunks
JC = S // 128     # 8 j chunks
NPAIR = 128       # query pairs per core: (i, i+128)
PHALF = 64        # pairs per psum half-batch

BF16 = mybir.dt.bfloat16
F32 = mybir.dt.float32

_NC_CACHE = {}


def build_nc(repeats: int = 1, phase: str = "full"):
    key = (repeats, phase)
    if key in _NC_CACHE:
        return _NC_CACHE[key]

    nc = bacc.Bacc("TRN2", target_bir_lowering=False, debug=False, num_devices=NC)

    hT = nc.dram_tensor("hT", [H, S], BF16, kind="ExternalInput")
    hqT = nc.dram_tensor("hqT", [H, SQ], BF16, kind="ExternalInput")
    wkT = nc.dram_tensor("wkT", [H, H], BF16, kind="ExternalInput")
    wvT = nc.dram_tensor("wvT", [H, H], BF16, kind="ExternalInput")
    wqT = nc.dram_tensor("wqT", [H, H], BF16, kind="ExternalInput")
    bkP = nc.dram_tensor("bkP", [128, HC], F32, kind="ExternalInput")
    bqP = nc.dram_tensor("bqP", [128, HC], F32, kind="ExternalInput")
    bvR = nc.dram_tensor("bvR", [1, H], F32, kind="ExternalInput")
    maskP = nc.dram_tensor("maskP", [128, JC], F32, kind="ExternalInput")
    bb4 = nc.dram_tensor("bb4", [JC, 128, NPAIR, 128], BF16, kind="ExternalInput")
    out = nc.dram_tensor("out", [SQ, H], F32, kind="ExternalOutput")

    with tile.TileContext(nc) as tc:
        if repeats == 1:
            _emit(nc, tc, hT, hqT, wkT, wvT, wqT, bkP, bqP, bvR, maskP, bb4, out,
                  phase)
        else:
            with tc.For_i(0, repeats, 1):
                _emit(nc, tc, hT, hqT, wkT, wvT, wqT, bkP, bqP, bvR, maskP, bb4,
                      out, phase)
    nc.compile()
    _NC_CACHE[key] = nc
    return nc


def _emit(nc, tc, hT, hqT, wkT, wvT, wqT, bkP, bqP, bvR, maskP, bb4, out,
          phase="full"):
    from contextlib import ExitStack

    do_proj = phase in ("proj", "scores", "full")
    do_scores = phase in ("scores", "full")
    do_pv = phase == "full"

    with ExitStack() as ctx:
        persist = ctx.enter_context(tc.tile_pool(name="persist", bufs=1))

        # long-lived tensors
        kT64 = persist.tile([64, NH, S], BF16)          # kT64[d, n, j] = K[j, 64n+d]
        qTD = persist.tile([128, NH, SQ], BF16)         # duplicated Q^T (both halves)
        LH = persist.tile([128, NPAIR, 24], BF16)       # block-diag Q stationaries
        v_sb = persist.tile([128, JC, NH, DH + 1], BF16)  # V + ones column, per head
        expt = persist.tile([128, NH, JC, SQ], BF16)    # exp(scores^T)
        ctx_sb = persist.tile([128, NH, 2, DH + 1], F32)  # ctx accumulator + denom
        out_sb = persist.tile([128, 2, H], F32)
        maskP_sb = persist.tile([128, JC], F32)
        nc.scalar.dma_start(maskP_sb[:], maskP[:])
        if not do_pv:
            nc.vector.memset(out_sb[:], 0.0)

        # ---------------- projections ----------------
        with ExitStack() as proj_ctx:
            consts = proj_ctx.enter_context(tc.tile_pool(name="consts", bufs=1))
            stage = proj_ctx.enter_context(tc.tile_pool(name="stage", bufs=1))
            ppsum = proj_ctx.enter_context(
                tc.tile_pool(name="ppsum", bufs=3, space="PSUM")
            )

            hT_sb = consts.tile([128, HC, S], BF16)
            nc.scalar.dma_start(hT_sb[:], hT.rearrange("(c p) s -> p c s", p=128))
            hqT_sb = consts.tile([128, HC, SQ], BF16)
            nc.scalar.dma_start(hqT_sb[:], hqT.rearrange("(c p) s -> p c s", p=128))
            wkT_sb = consts.tile([128, HC, H], BF16)
            nc.scalar.dma_start(wkT_sb[:], wkT.rearrange("(c p) o -> p c o", p=128))
            wvT_sb = consts.tile([128, HC, H], BF16)
            nc.scalar.dma_start(wvT_sb[:], wvT.rearrange("(c p) o -> p c o", p=128))
            wqT_sb = consts.tile([128, HC, H], BF16)
            nc.scalar.dma_start(wqT_sb[:], wqT.rearrange("(c p) o -> p c o", p=128))
            bkP_sb = consts.tile([128, HC], F32)
            nc.scalar.dma_start(bkP_sb[:], bkP[:])
            bqP_sb = consts.tile([128, HC], F32)
            nc.scalar.dma_start(bqP_sb[:], bqP[:])
            bvR_sb = consts.tile([128, H], F32)
            nc.gpsimd.dma_start(bvR_sb[:], bvR[:].to_broadcast((128, H)))

            kT_sb = stage.tile([128, HC, S], BF16)      # K^T in feature-chunk layout
            qT_sb = stage.tile([128, HC, SQ], BF16)

            # K^T: out[o-chunk, s] ; accumulate over input chunks
            for c in range(HC) if do_proj else []:
                for jb in range(2):
                    ps = ppsum.tile([128, 512], F32, tag="projps")
                    for ci in range(HC):
                        nc.tensor.matmul(
                            ps[:],
                            wkT_sb[:, ci, c * 128 : (c + 1) * 128],
                            hT_sb[:, ci, jb * 512 : (jb + 1) * 512],
                            start=(ci == 0),
                            stop=(ci == HC - 1),
                        )
                    nc.scalar.activation(
                        out=kT_sb[:, c, jb * 512 : (jb + 1) * 512],
                        in_=ps[:],
                        func=mybir.ActivationFunctionType.Identity,
                        bias=bkP_sb[:, c : c + 1],
                    )

            # V: out[j-chunk, o]; bias added along free dim via DVE
            for sc in range(JC) if do_proj else []:
                for ob in range(2):
                    ps = ppsum.tile([128, 384], F32, tag="projps")
                    for ci in range(HC):
                        nc.tensor.matmul(
                            ps[:],
                            hT_sb[:, ci, sc * 128 : (sc + 1) * 128],
                            wvT_sb[:, ci, ob * 384 : (ob + 1) * 384],
                            start=(ci == 0),
                            stop=(ci == HC - 1),
                        )
                    nc.vector.tensor_add(
                        out=v_sb[:, sc, ob * 6 : (ob + 1) * 6, 0:DH],
                        in0=ps[:].rearrange("p (n d) -> p n d", d=DH),
                        in1=bvR_sb[:, ob * 384 : (ob + 1) * 384].rearrange(
                            "p (n d) -> p n d", d=DH
                        ),
                    )
            # ones column for the softmax denominator
            if do_proj:
                nc.vector.memset(v_sb[:, :, :, DH : DH + 1], 1.0)

            # Q^T
            for c in range(HC) if do_proj else []:
                ps = ppsum.tile([128, SQ], F32, tag="projps")
                for ci in range(HC):
                    nc.tensor.matmul(
                        ps[:],
                        wqT_sb[:, ci, c * 128 : (c + 1) * 128],
                        hqT_sb[:, ci, :],
                        start=(ci == 0),
                        stop=(ci == HC - 1),
                    )
                nc.scalar.activation(
                    out=qT_sb[:, c, :],
                    in_=ps[:],
                    func=mybir.ActivationFunctionType.Identity,
                    bias=bqP_sb[:, c : c + 1],
                )

            # rearrange to per-head 64-partition layouts (SBUF->SBUF DMA)
            if do_proj:
                for n in range(NH):
                    src = kT_sb[64 * (n % 2) : 64 * (n % 2) + 64, n // 2, :]
                    nc.scalar.dma_start(kT64[:, n, :], src)
                for n in range(NH):
                    src = qT_sb[64 * (n % 2) : 64 * (n % 2) + 64, n // 2, :]
                    nc.scalar.dma_start(qTD[0:64, n, :], src)
                    nc.scalar.dma_start(qTD[64:128, n, :], src)

                # block-diagonal Q stationaries for the bbox matmuls
                nc.vector.memset(LH[:], 0.0)
                nc.vector.tensor_copy(
                    out=LH[0:64, :, 0:12].rearrange("p pr n -> p n pr"),
                    in_=qTD[0:64, :, 0:128],
                )
                nc.vector.tensor_copy(
                    out=LH[64:128, :, 12:24].rearrange("p pr n -> p n pr"),
                    in_=qTD[64:128, :, 128:256],
                )

        # ---------------- scores^T + exp, per j-chunk ----------------
        with ExitStack() as loop_ctx:
            bbsb = loop_ctx.enter_context(tc.tile_pool(name="bbsb", bufs=6))
            scorep = loop_ctx.enter_context(tc.tile_pool(name="scorep", bufs=2))
            bbps = loop_ctx.enter_context(
                tc.tile_pool(name="bbps", bufs=2, space="PSUM")
            )
            qktps = loop_ctx.enter_context(
                tc.tile_pool(name="qktps", bufs=2, space="PSUM")
            )
            pvps = loop_ctx.enter_context(
                tc.tile_pool(name="pvps", bufs=2, space="PSUM")
            )

            PQ = 32   # pairs per psum quarter-batch / DMA chunk
            HG = 2    # heads per qkt psum group
            for jc in range(JC):
                bbqs = []
                for q in range(NPAIR // PQ):
                    bbq = bbsb.tile([128, PQ, 128], BF16, tag="bbjc")
                    nc.sync.dma_start(bbq[:], bb4[jc][:, q * PQ : (q + 1) * PQ, :])
                    bbqs.append(bbq)
                if not do_scores:
                    continue

                scoresT = scorep.tile([128, NH, SQ], F32, tag="scoresT")
                for g in range(NH // HG):
                    qkt = qktps.tile([128, HG, SQ], F32, tag="qkt")
                    for k in range(HG):
                        n = g * HG + k
                        nc.tensor.matmul(
                            qkt[:, k, :],
                            kT64[:, n, jc * 128 : (jc + 1) * 128],
                            qTD[0:64, n, :],
                            start=True,
                            stop=True,
                        )
                    nc.scalar.copy(scoresT[:, g * HG : (g + 1) * HG, :], qkt[:])

                for quarter in range(NPAIR // PQ):
                    bbt = bbps.tile([128, PQ, 32], F32, tag="bbt")
                    for pl in range(PQ):
                        p = quarter * PQ + pl
                        nc.tensor.matmul(
                            bbt[:, pl, 0:24],
                            bbqs[quarter][:, pl, :],
                            LH[:, p, :],
                            start=True,
                            stop=True,
                        )
                    # scoresT[jj, n, 128s + 32*quarter + p] += bbt[jj, p, 12s + n]
                    dst = scoresT.rearrange("j n (s i) -> j n s i", s=2)[
                        :, :, :, quarter * PQ : (quarter + 1) * PQ
                    ]
                    src = bbt[:, :, 0:24].rearrange("j p (s n) -> j n s p", s=2)
                    nc.vector.tensor_add(out=dst, in0=dst, in1=src)

                nc.scalar.activation(
                    out=expt[:, :, jc, :],
                    in_=scoresT[:],
                    func=mybir.ActivationFunctionType.Exp,
                    bias=maskP_sb[:, jc : jc + 1],
                    scale=0.125,
                )

                # PV for this j-chunk, accumulated into SBUF (k = 2n + ic)
                if do_pv:
                    ctx_v = ctx_sb.rearrange("p n ic c -> p (n ic) c")
                    for batch in range(24 // 4):
                        pvp = pvps.tile([128, 4, 128], F32, tag="pv")
                        for kk in range(4):
                            k = batch * 4 + kk
                            n, ic = divmod(k, 2)
                            nc.tensor.matmul(
                                pvp[:, kk, 0 : DH + 1],
                                expt[:, n, jc, ic * 128 : (ic + 1) * 128],
                                v_sb[:, jc, n, :],
                                start=True,
                                stop=True,
                            )
                        dst = ctx_v[:, batch * 4 : (batch + 1) * 4, :]
                        if jc == 0:
                            nc.vector.tensor_copy(out=dst, in_=pvp[:, :, 0 : DH + 1])
                        else:
                            nc.vector.tensor_add(
                                out=dst, in0=dst, in1=pvp[:, :, 0 : DH + 1]
                            )

        # ---------------- normalize ----------------
        if do_pv:
            with tc.tile_pool(name="sm", bufs=1) as sm:
                rec = sm.tile([128, NH, 2, 1], F32)
                nc.vector.reciprocal(out=rec[:], in_=ctx_sb[:, :, :, DH : DH + 1])
                nc.vector.tensor_mul(
                    out=out_sb.rearrange("p ic (n d) -> p n ic d", d=DH),
                    in0=ctx_sb[:, :, :, 0:DH],
                    in1=rec[:].to_broadcast((128, NH, 2, DH)),
                )

        nc.scalar.dma_start(out.rearrange("(ic p) h -> p ic h", p=128), out_sb[:])


def _prep_core_inputs(hidden, bbox, mask, wkT, wvT, wqT, bkP, bqP, bvR):
    """Build the 8 per-core input dicts (host-side shard + layout prep)."""
    in_maps = []
    for core in range(NC):
        b, q4 = divmod(core, 4)
        i0 = q4 * SQ
        hTb = np.ascontiguousarray(hidden[b].T).astype(ml_dtypes.bfloat16)
        X = bbox[i0 : i0 + SQ, :, b, :]  # (256, 1024, 64)
        bb4 = np.ascontiguousarray(
            X.reshape(2, 128, JC, 128, DH).transpose(2, 0, 4, 1, 3)
        ).reshape(JC, 128, NPAIR, 128).astype(ml_dtypes.bfloat16)
        maskPc = np.ascontiguousarray(
            mask[b, 0, 0, :].reshape(JC, 128).T
        ).astype(np.float32)
        in_maps.append(
            {
                "hT": hTb,
                "hqT": np.ascontiguousarray(hTb[:, i0 : i0 + SQ]),
                "wkT": wkT,
                "wvT": wvT,
                "wqT": wqT,
                "bkP": bkP,
                "bqP": bqP,
                "bvR": bvR,
                "maskP": maskPc,
                "bb4": bb4,
            }
        )
    return in_maps


def _prep_shared(Wq, bq, Wk, bk, Wv, bv):
    wkT = np.ascontiguousarray(Wk.T).astype(ml_dtypes.bfloat16)
    wvT = np.ascontiguousarray(Wv.T).astype(ml_dtypes.bfloat16)
    wqT = np.ascontiguousarray(Wq.T).astype(ml_dtypes.bfloat16)
    bkP = np.ascontiguousarray(bk.reshape(HC, 128).T).astype(np.float32)
    bqP = np.ascontiguousarray(bq.reshape(HC, 128).T).astype(np.float32)
    bvR = bv.reshape(1, H).astype(np.float32)
    return wkT, wvT, wqT, bkP, bqP, bvR


def kernel(hidden_states, bbox_pos_emb, attention_mask, Wq, bq, Wk, bk, Wv, bv):
    hidden = np.asarray(hidden_states, dtype=np.float32)
    bbox = np.asarray(bbox_pos_emb, dtype=np.float32)
    mask = np.asarray(attention_mask, dtype=np.float32)
    shared = _prep_shared(
        np.asarray(Wq, np.float32),
        np.asarray(bq, np.float32),
        np.asarray(Wk, np.float32),
        np.asarray(bk, np.float32),
        np.asarray(Wv, np.float32),
        np.asarray(bv, np.float32),
    )
    in_maps = _prep_core_inputs(hidden, bbox, mask, *shared)
    nc = build_nc()
    res = run_bass_kernel_spmd(nc, in_maps, core_ids=list(range(NC)))
    full = np.empty((B, S, H), np.float32)
    for core in range(NC):
        b, q4 = divmod(core, 4)
        full[b, q4 * SQ : (q4 + 1) * SQ, :] = res.results[core]["out"]
    return full



# revision 2
# speedup vs baseline: 1.5425x; 1.5425x over previous
"""BrosSelfAttention Trainium2 kernel (optimized).

Problem: B=2, S=1024, H=768, NH=12, DH=64.
  q,k,v = proj(hidden);  scores = q@k.T + einsum('bnid,bijd->bnij', q, bbox)
  probs = softmax(scores/8 + mask);  ctx = probs@v

Sharding: 8 cores = (batch b in {0,1}) x (query block q4 in {0..3}, 256 rows).
Each core computes ctx for its 256 query rows, all 12 heads. K/V are
recomputed per core (full S for its batch).

Layout trick: scores are computed TRANSPOSED ([j, i] with j on partitions)
so that
  - the bbox term is computed as per-query-pair matmuls
    (stationary = bbox slice [sd=128, jj=128], moving = block-diag Q [128, 24])
    whose output [jj, (s,n)] has j on partitions,
  - the QK term ([jj, i] = kT_n^T @ qT_n) also has j on partitions,
  - the attention mask becomes a per-partition bias folded into the exp
    activation (exact reference semantics),
  - softmax row sums fall out of the PV matmul via a ones-column in V,
  - the PV matmul (lhsT = exp(scores^T) [jj, i], rhs = V [jj, 65]) contracts
    over j and lands ctx in [i, dh] layout directly.

All matmuls bf16 inputs with fp32 PSUM accumulation.
"""

import numpy as np
import ml_dtypes

import concourse.mybir as mybir
import concourse.tile as tile
from concourse import bacc
from concourse.bass_utils import run_bass_kernel_spmd

B, S, H, NH, DH = 2, 1024, 768, 12, 64
SQ = 256          # query rows per core
NC = 8            # cores
HC = H // 128     # 6 feature chunks
JC = S // 128     # 8 j chunks
NPAIR = 128       # query pairs per core: (i, i+128)
PHALF = 64        # pairs per psum half-batch

BF16 = mybir.dt.bfloat16
F32 = mybir.dt.float32
FP8 = mybir.dt.float8e4

_NC_CACHE = {}


def build_nc(repeats: int = 1, phase: str = "full"):
    key = (repeats, phase)
    if key in _NC_CACHE:
        return _NC_CACHE[key]

    nc = bacc.Bacc("TRN2", target_bir_lowering=False, debug=False, num_devices=NC)

    hT = nc.dram_tensor("hT", [H, S], BF16, kind="ExternalInput")
    hqT = nc.dram_tensor("hqT", [H, SQ], BF16, kind="ExternalInput")
    wkT = nc.dram_tensor("wkT", [H, H], BF16, kind="ExternalInput")
    wvT = nc.dram_tensor("wvT", [H, H], BF16, kind="ExternalInput")
    wqT = nc.dram_tensor("wqT", [H, H], BF16, kind="ExternalInput")
    bkP = nc.dram_tensor("bkP", [128, HC], F32, kind="ExternalInput")
    bqP = nc.dram_tensor("bqP", [128, HC], F32, kind="ExternalInput")
    bvR = nc.dram_tensor("bvR", [1, H], F32, kind="ExternalInput")
    maskP = nc.dram_tensor("maskP", [128, JC], F32, kind="ExternalInput")
    bb4 = nc.dram_tensor("bb4", [JC, 128, NPAIR, 128], FP8, kind="ExternalInput")
    out = nc.dram_tensor("out", [SQ, H], F32, kind="ExternalOutput")

    with tile.TileContext(nc) as tc:
        if repeats == 1:
            _emit(nc, tc, hT, hqT, wkT, wvT, wqT, bkP, bqP, bvR, maskP, bb4, out,
                  phase)
        else:
            with tc.For_i(0, repeats, 1):
                _emit(nc, tc, hT, hqT, wkT, wvT, wqT, bkP, bqP, bvR, maskP, bb4,
                      out, phase)
    nc.compile()
    _NC_CACHE[key] = nc
    return nc


def _emit(nc, tc, hT, hqT, wkT, wvT, wqT, bkP, bqP, bvR, maskP, bb4, out,
          phase="full"):
    from contextlib import ExitStack

    do_proj = phase in ("proj", "scores", "full", "nobb")
    do_scores = phase in ("scores", "full", "nobb")
    do_bb = phase != "nobb"
    do_pv = phase in ("full", "nobb")

    with ExitStack() as ctx:
        persist = ctx.enter_context(tc.tile_pool(name="persist", bufs=1))

        # long-lived tensors
        # kT2[0:64, hp, j] = K[j, head 2hp dims], kT2[64:128, hp, j] = head 2hp+1
        # (this IS the proj output layout -- no rearrange needed)
        kT2 = persist.tile([128, HC, S], BF16)
        qTD = persist.tile([128, NH, SQ], BF16)         # duplicated Q^T (both halves)
        LH = persist.tile([128, NPAIR, 24], BF16)       # block-diag Q stationaries
        QD2 = persist.tile([128, HC, 2, SQ], BF16)      # block-diag Q for 2-head QK
        v_sb = persist.tile([128, JC, NH, DH + 1], BF16)  # V + ones column, per head
        ctx_sb = persist.tile([128, NH, 2, DH + 1], F32)  # ctx accumulator + denom
        out_sb = persist.tile([128, 2, H], F32)
        maskP_sb = persist.tile([128, JC], F32)
        nc.scalar.dma_start(maskP_sb[:], maskP[:])
        if not do_pv:
            nc.vector.memset(out_sb[:], 0.0)

        # bbox staging pool: allocated BEFORE the proj pools so its SBUF
        # addresses never alias proj tiles -- otherwise WAR deps on the
        # reused address range stall the bbox DMA stream until proj dies.
        # fp8 bbox: one 2MB DMA per j-chunk.
        bbsb = ctx.enter_context(tc.tile_pool(name="bbsb", bufs=5))
        PQ = 32   # pairs per psum quarter-batch
        all_bbqs = []
        for jc in range(JC):
            bbq = bbsb.tile([128, NPAIR, 128], FP8, tag="bbjc")
            nc.sync.dma_start(bbq[:], bb4[jc][:])
            all_bbqs.append(bbq)

        # ---------------- projections ----------------
        with ExitStack() as proj_ctx:
            consts = proj_ctx.enter_context(tc.tile_pool(name="consts", bufs=1))
            stage = proj_ctx.enter_context(tc.tile_pool(name="stage", bufs=1))
            ppsum = proj_ctx.enter_context(
                tc.tile_pool(name="ppsum", bufs=3, space="PSUM")
            )

            hT_sb = consts.tile([128, HC, S], BF16)
            nc.scalar.dma_start(hT_sb[:], hT.rearrange("(c p) s -> p c s", p=128))
            hqT_sb = consts.tile([128, HC, SQ], BF16)
            nc.scalar.dma_start(hqT_sb[:], hqT.rearrange("(c p) s -> p c s", p=128))
            wkT_sb = consts.tile([128, HC, H], BF16)
            nc.scalar.dma_start(wkT_sb[:], wkT.rearrange("(c p) o -> p c o", p=128))
            wvT_sb = consts.tile([128, HC, H], BF16)
            nc.scalar.dma_start(wvT_sb[:], wvT.rearrange("(c p) o -> p c o", p=128))
            wqT_sb = consts.tile([128, HC, H], BF16)
            nc.scalar.dma_start(wqT_sb[:], wqT.rearrange("(c p) o -> p c o", p=128))
            bkP_sb = consts.tile([128, HC], F32)
            nc.scalar.dma_start(bkP_sb[:], bkP[:])
            bqP_sb = consts.tile([128, HC], F32)
            nc.scalar.dma_start(bqP_sb[:], bqP[:])
            bvR_sb = consts.tile([128, H], F32)
            nc.gpsimd.dma_start(bvR_sb[:], bvR[:].to_broadcast((128, H)))

            qT_sb = stage.tile([128, HC, SQ], BF16)

            # Q^T first: the bbox matmuls only need LH (built from Q), so
            # putting Q-proj first lets them start while K/V proj still runs.
            for c in range(HC) if do_proj else []:
                ps = ppsum.tile([128, SQ], F32, tag="projps")
                for ci in range(HC):
                    nc.tensor.matmul(
                        ps[:],
                        wqT_sb[:, ci, c * 128 : (c + 1) * 128],
                        hqT_sb[:, ci, :],
                        start=(ci == 0),
                        stop=(ci == HC - 1),
                    )
                nc.scalar.activation(
                    out=qT_sb[:, c, :],
                    in_=ps[:],
                    func=mybir.ActivationFunctionType.Identity,
                    bias=bqP_sb[:, c : c + 1],
                )

            if do_proj:
                for n in range(NH):
                    src = qT_sb[64 * (n % 2) : 64 * (n % 2) + 64, n // 2, :]
                    nc.scalar.dma_start(qTD[0:64, n, :], src)
                    nc.scalar.dma_start(qTD[64:128, n, :], src)

                # block-diagonal Q stationaries for the bbox matmuls
                nc.vector.memset(LH[:], 0.0)
                nc.vector.tensor_copy(
                    out=LH[0:64, :, 0:12].rearrange("p pr n -> p n pr"),
                    in_=qTD[0:64, :, 0:128],
                )
                nc.vector.tensor_copy(
                    out=LH[64:128, :, 12:24].rearrange("p pr n -> p n pr"),
                    in_=qTD[64:128, :, 128:256],
                )

                # block-diagonal Q moving operand for 2-head-packed QK:
                # QD2[0:64, hp, 0, i] = q[i, head 2hp, :], QD2[64:128, hp, 1, i]
                # = q[i, head 2hp+1, :], zeros elsewhere.
                nc.vector.memset(QD2[:], 0.0)
                for hp in range(HC):
                    nc.vector.tensor_copy(
                        out=QD2[0:64, hp, 0, :], in_=qTD[0:64, 2 * hp, :]
                    )
                    nc.vector.tensor_copy(
                        out=QD2[64:128, hp, 1, :], in_=qTD[64:128, 2 * hp + 1, :]
                    )

            # K^T: out[o-chunk, s] ; accumulate over input chunks
            for c in range(HC) if do_proj else []:
                for jb in range(2):
                    ps = ppsum.tile([128, 512], F32, tag="projps")
                    for ci in range(HC):
                        nc.tensor.matmul(
                            ps[:],
                            wkT_sb[:, ci, c * 128 : (c + 1) * 128],
                            hT_sb[:, ci, jb * 512 : (jb + 1) * 512],
                            start=(ci == 0),
                            stop=(ci == HC - 1),
                        )
                    nc.scalar.activation(
                        out=kT2[:, c, jb * 512 : (jb + 1) * 512],
                        in_=ps[:],
                        func=mybir.ActivationFunctionType.Identity,
                        bias=bkP_sb[:, c : c + 1],
                    )

            # V: out[j-chunk, o]; bias added along free dim via DVE
            for sc in range(JC) if do_proj else []:
                for ob in range(2):
                    ps = ppsum.tile([128, 384], F32, tag="projps")
                    for ci in range(HC):
                        nc.tensor.matmul(
                            ps[:],
                            hT_sb[:, ci, sc * 128 : (sc + 1) * 128],
                            wvT_sb[:, ci, ob * 384 : (ob + 1) * 384],
                            start=(ci == 0),
                            stop=(ci == HC - 1),
                        )
                    nc.vector.tensor_add(
                        out=v_sb[:, sc, ob * 6 : (ob + 1) * 6, 0:DH],
                        in0=ps[:].rearrange("p (n d) -> p n d", d=DH),
                        in1=bvR_sb[:, ob * 384 : (ob + 1) * 384].rearrange(
                            "p (n d) -> p n d", d=DH
                        ),
                    )
            # ones column for the softmax denominator
            if do_proj:
                nc.vector.memset(v_sb[:, :, :, DH : DH + 1], 1.0)

        # ---------------- scores^T + exp, per j-chunk ----------------
        with ExitStack() as loop_ctx:
            scorep = loop_ctx.enter_context(tc.tile_pool(name="scorep", bufs=2))
            probsp = loop_ctx.enter_context(tc.tile_pool(name="probsp", bufs=3))
            bbps = loop_ctx.enter_context(
                tc.tile_pool(name="bbps", bufs=2, space="PSUM")
            )
            qktps = loop_ctx.enter_context(
                tc.tile_pool(name="qktps", bufs=2, space="PSUM")
            )
            pvps = loop_ctx.enter_context(
                tc.tile_pool(name="pvps", bufs=2, space="PSUM")
            )

            HG = 2    # heads per qkt psum group
            for jc in range(JC):
                bbqs = all_bbqs[jc]
                if not do_scores:
                    continue

                scoresT = scorep.tile([128, NH, SQ], F32, tag="scoresT")
                for hp in range(HC):
                    qkt = qktps.tile([128, HG, SQ], F32, tag="qkt")
                    nc.tensor.matmul(
                        qkt[:].rearrange("j h i -> j (h i)"),
                        kT2[:, hp, jc * 128 : (jc + 1) * 128],
                        QD2[:, hp].rearrange("p t i -> p (t i)"),
                        start=True,
                        stop=True,
                    )
                    nc.scalar.copy(scoresT[:, hp * HG : (hp + 1) * HG, :], qkt[:])

                for quarter in range(NPAIR // PQ) if do_bb else []:
                    bbt = bbps.tile([128, PQ, 32], F32, tag="bbt")
                    for pl in range(PQ):
                        p = quarter * PQ + pl
                        nc.tensor.matmul(
                            bbt[:, pl, 0:24],
                            bbqs[:, p, :],
                            LH[:, p, :],
                            start=True,
                            stop=True,
                        )
                    # scoresT[jj, n, 128s + 32*quarter + p] += bbt[jj, p, 12s + n]
                    dst = scoresT.rearrange("j n (s i) -> j n s i", s=2)[
                        :, :, :, quarter * PQ : (quarter + 1) * PQ
                    ]
                    src = bbt[:, :, 0:24].rearrange("j p (s n) -> j n s p", s=2)
                    nc.vector.tensor_add(out=dst, in0=dst, in1=src)

                probs = probsp.tile([128, NH, SQ], BF16, tag="probs")
                nc.scalar.activation(
                    out=probs[:],
                    in_=scoresT[:],
                    func=mybir.ActivationFunctionType.Exp,
                    bias=maskP_sb[:, jc : jc + 1],
                    scale=0.125,
                )

                # PV for this j-chunk, accumulated into SBUF (k = 2n + ic)
                if do_pv:
                    ctx_v = ctx_sb.rearrange("p n ic c -> p (n ic) c")
                    for batch in range(24 // 4):
                        pvp = pvps.tile([128, 4, 128], F32, tag="pv")
                        for kk in range(4):
                            k = batch * 4 + kk
                            n, ic = divmod(k, 2)
                            nc.tensor.matmul(
                                pvp[:, kk, 0 : DH + 1],
                                probs[:, n, ic * 128 : (ic + 1) * 128],
                                v_sb[:, jc, n, :],
                                start=True,
                                stop=True,
                            )
                        dst = ctx_v[:, batch * 4 : (batch + 1) * 4, :]
                        if jc == 0:
                            nc.vector.tensor_copy(out=dst, in_=pvp[:, :, 0 : DH + 1])
                        else:
                            nc.vector.tensor_add(
                                out=dst, in0=dst, in1=pvp[:, :, 0 : DH + 1]
                            )

        # ---------------- normalize ----------------
        if do_pv:
            with tc.tile_pool(name="sm", bufs=1) as sm:
                rec = sm.tile([128, NH, 2, 1], F32)
                nc.vector.reciprocal(out=rec[:], in_=ctx_sb[:, :, :, DH : DH + 1])
                nc.vector.tensor_mul(
                    out=out_sb.rearrange("p ic (n d) -> p n ic d", d=DH),
                    in0=ctx_sb[:, :, :, 0:DH],
                    in1=rec[:].to_broadcast((128, NH, 2, DH)),
                )

        nc.scalar.dma_start(out.rearrange("(ic p) h -> p ic h", p=128), out_sb[:])


def _prep_core_inputs(hidden, bbox, mask, wkT, wvT, wqT, bkP, bqP, bvR):
    """Build the 8 per-core input dicts (host-side shard + layout prep)."""
    in_maps = []
    for core in range(NC):
        b, q4 = divmod(core, 4)
        i0 = q4 * SQ
        hTb = np.ascontiguousarray(hidden[b].T).astype(ml_dtypes.bfloat16)
        X = bbox[i0 : i0 + SQ, :, b, :]  # (256, 1024, 64)
        bb4 = np.ascontiguousarray(
            X.reshape(2, 128, JC, 128, DH).transpose(2, 0, 4, 1, 3)
        ).reshape(JC, 128, NPAIR, 128).astype(ml_dtypes.float8_e4m3)
        maskPc = np.ascontiguousarray(
            mask[b, 0, 0, :].reshape(JC, 128).T
        ).astype(np.float32)
        in_maps.append(
            {
                "hT": hTb,
                "hqT": np.ascontiguousarray(hTb[:, i0 : i0 + SQ]),
                "wkT": wkT,
                "wvT": wvT,
                "wqT": wqT,
                "bkP": bkP,
                "bqP": bqP,
                "bvR": bvR,
                "maskP": maskPc,
                "bb4": bb4,
            }
        )
    return in_maps


def _prep_shared(Wq, bq, Wk, bk, Wv, bv):
    wkT = np.ascontiguousarray(Wk.T).astype(ml_dtypes.bfloat16)
    wvT = np.ascontiguousarray(Wv.T).astype(ml_dtypes.bfloat16)
    wqT = np.ascontiguousarray(Wq.T).astype(ml_dtypes.bfloat16)
    bkP = np.ascontiguousarray(bk.reshape(HC, 128).T).astype(np.float32)
    bqP = np.ascontiguousarray(bq.reshape(HC, 128).T).astype(np.float32)
    bvR = bv.reshape(1, H).astype(np.float32)
    return wkT, wvT, wqT, bkP, bqP, bvR


def kernel(hidden_states, bbox_pos_emb, attention_mask, Wq, bq, Wk, bk, Wv, bv):
    hidden = np.asarray(hidden_states, dtype=np.float32)
    bbox = np.asarray(bbox_pos_emb, dtype=np.float32)
    mask = np.asarray(attention_mask, dtype=np.float32)
    shared = _prep_shared(
        np.asarray(Wq, np.float32),
        np.asarray(bq, np.float32),
        np.asarray(Wk, np.float32),
        np.asarray(bk, np.float32),
        np.asarray(Wv, np.float32),
        np.asarray(bv, np.float32),
    )
    in_maps = _prep_core_inputs(hidden, bbox, mask, *shared)
    nc = build_nc()
    res = run_bass_kernel_spmd(nc, in_maps, core_ids=list(range(NC)))
    full = np.empty((B, S, H), np.float32)
    for core in range(NC):
        b, q4 = divmod(core, 4)
        full[b, q4 * SQ : (q4 + 1) * SQ, :] = res.results[core]["out"]
    return full

